# revision 7
# baseline (speedup 1.0000x reference)
"""Trainium2 Bass kernel for a 2-layer GCN with attention-weighted periods.

Reference computation (PERIODS=1, so softmax(attention)=1.0 exactly):
    h  = relu(gcn_conv(x, W1, b1))
    out = softmax(gcn_conv(h, W2, b2), axis=1)
where gcn_conv adds self-loops (weight 1), applies symmetric normalization
norm_e = dinv[src]*w_e*dinv[dst] with deg[j] = 1 + sum_{e->j} w_e, and
aggregates  out[d] = sum_e norm_e * (x @ W)[src_e] + b.

Key algebraic facts exploited:
  * aggregation commutes with the dense matmul:
        sum_e norm_e * (x@W)[src] = (sum_e norm_e * x[src]) @ W
    so the gather tables are the raw inputs (x for layer 1, h@W2 for layer 2)
    and the dense matmuls run once per 128 destination nodes.
  * both layers share the same normalized adjacency, computed once on the
    host as graph preprocessing (cached gcn_norm), and the same edge
    partitioning metadata.

Distribution (8 NeuronCores): nodes are sharded contiguously (12544/core);
edges are partitioned by destination core and grouped into 128-destination
"windows" that accumulate in PSUM via one-hot-valued matmuls
(S[e, d] = norm_e * (local_dst[e] == d)); source features are fetched with
dma_gather from a replicated table. Layer 2's table (h@W2, padded to 64
cols) is exchanged with an on-device AllGather.
"""

import math
import numpy as np

# ---------------------------------------------------------------- problem cfg
N = 100000          # real nodes
F = 128             # input/hidden features
C = 40              # classes
CORES = 8
NPC = 12544         # nodes per core (multiple of 128)
NPAD = CORES * NPC  # 100352
WPC = NPC // 128    # windows per core = 98
NCH = 4             # source chunks (int16 gather index limit: 32767 rows)
CHUNK = NPAD // NCH  # 25088 rows per chunk
HWC = 64            # layer-2 table row width (256B gather granularity)
QUANT = 64          # quantize per-call index counts (register-value reuse)

_CACHE = {}


def _ceil_to(a, m):
    return -(-a // m) * m


# ------------------------------------------------------------------ host prep
def _host_prep(x, edge_index, edge_weight):
    """Builds per-core gather/one-hot metadata from the edge list.

    Returns (x_pad, per_core list of dicts, cpad [G] int array, tcs).
    """
    row = np.asarray(edge_index[0], dtype=np.int64)
    col = np.asarray(edge_index[1], dtype=np.int64)
    w = np.asarray(edge_weight, dtype=np.float64)

    deg = np.bincount(col, weights=w, minlength=N) + 1.0
    dinv = 1.0 / np.sqrt(deg)
    coef = dinv[row] * w * dinv[col]

    loop = np.arange(N, dtype=np.int64)
    arow = np.concatenate([row, loop])
    acol = np.concatenate([col, loop])
    acoef = np.concatenate([coef, dinv * dinv]).astype(np.float32)

    core = acol // NPC
    G = WPC * NCH

    per_core = []
    counts = np.zeros((CORES, G), dtype=np.int64)
    for p in range(CORES):
        sel = core == p
        r = arow[sel]
        cl = acol[sel] - p * NPC
        q = acoef[sel]
        wid = cl >> 7
        ld = (cl & 127).astype(np.float32)
        ch = r // CHUNK
        li = (r - ch * CHUNK).astype(np.int64)
        grp = wid * NCH + ch
        order = np.lexsort((r, grp))
        grp, ld, li, q = grp[order], ld[order], li[order], q[order]
        counts[p] = np.bincount(grp, minlength=G)
        per_core.append({"grp": grp, "ld": ld, "li": li, "q": q})

    cmax = counts.max(axis=0)
    cpad = np.minimum(_ceil_to(1, QUANT) * 0 + ((cmax + QUANT - 1) // QUANT) * QUANT,
                      10 ** 9)
    tcs = max(1, int(-(-int(cpad.max()) // 128)))
    cap = tcs * 128
    assert (cpad <= cap).all()

    idxb = tcs * 8          # idx columns per group
    slot_cols = G * tcs     # ld/q columns

    for p in range(CORES):
        d = per_core[p]
        grp, ld, li, q = d["grp"], d["ld"], d["li"], d["q"]
        cnt = counts[p]
        starts = np.zeros(G + 1, dtype=np.int64)
        np.cumsum(cnt, out=starts[1:])
        pos = np.arange(len(grp)) - starts[grp]

        idx_arr = np.full((G, cap), -1, dtype=np.int16)
        idx_arr[grp, pos] = li.astype(np.int16)
        ar = np.arange(cap)
        padmask = (ar[None, :] >= cnt[:, None]) & (ar[None, :] < cpad[:, None])
        idx_arr[padmask] = 0

        ld_arr = np.zeros((G, cap), dtype=np.float32)
        q_arr = np.zeros((G, cap), dtype=np.float32)
        ld_arr[grp, pos] = ld
        q_arr[grp, pos] = q

        idx_dev = idx_arr.reshape(G, idxb, 16).transpose(2, 0, 1).reshape(16, G * idxb)
        idx_dev = np.ascontiguousarray(np.tile(idx_dev, (8, 1)))
        ld_dev = np.ascontiguousarray(ld_arr.reshape(G * tcs, 128).T)
        q_dev = np.ascontiguousarray(q_arr.reshape(G * tcs, 128).T)
        per_core[p] = {"idx": idx_dev, "ld": ld_dev, "q": q_dev}

    x = np.asarray(x, dtype=np.float32)
    x_pad = np.zeros((NPAD, F), dtype=np.float32)
    x_pad[:N] = x
    return x_pad, per_core, cpad.astype(np.int64), tcs, slot_cols, idxb


# ------------------------------------------------------------------- builder
def _build(cpad, tcs, slot_cols, idxb):
    from contextlib import ExitStack
    import concourse.bacc as bacc
    import concourse.tile as tile
    import concourse.mybir as mybir
    from concourse.masks import make_identity

    f32 = mybir.dt.float32
    i16 = mybir.dt.int16
    i32 = mybir.dt.int32
    G = WPC * NCH
    IDX_COLS = G * idxb
    SLOTS = NCH * tcs

    nc = bacc.Bacc("TRN2", target_bir_lowering=False)

    x_ext = nc.declare_dram_parameter("x", [NPAD, F], f32, isOutput=False)
    idx_ext = nc.declare_dram_parameter("idx", [128, IDX_COLS], i16, isOutput=False)
    ld_ext = nc.declare_dram_parameter("ld", [128, slot_cols], f32, isOutput=False)
    q_ext = nc.declare_dram_parameter("q", [128, slot_cols], f32, isOutput=False)
    w1_ext = nc.declare_dram_parameter("W1", [F, F], f32, isOutput=False)
    b1_ext = nc.declare_dram_parameter("b1", [F], f32, isOutput=False)
    w2_ext = nc.declare_dram_parameter("W2", [F, C], f32, isOutput=False)
    b2_ext = nc.declare_dram_parameter("b2", [C], f32, isOutput=False)
    out_ext = nc.declare_dram_parameter("out", [NPC, C], f32, isOutput=True)

    hw_own = nc.dram_tensor("hw_own", [NPC, HWC], f32)
    hw_tab = nc.dram_tensor("hw_tab", [NPAD, HWC], f32, addr_space="Shared")

    tgs = [max(0, -(-int(cpad[g]) // 128)) if cpad[g] > 0 else 0 for g in range(G)]

    eq = mybir.AluOpType.is_equal
    mul = mybir.AluOpType.mult
    add = mybir.AluOpType.add
    mx_op = mybir.AluOpType.max
    AX = mybir.AxisListType.X
    Relu = mybir.ActivationFunctionType.Relu
    Exp = mybir.ActivationFunctionType.Exp

    with ExitStack() as ctx:
        tc = ctx.enter_context(tile.TileContext(nc))
        const = ctx.enter_context(tc.tile_pool(name="const", bufs=1))
        meta = ctx.enter_context(tc.tile_pool(name="meta", bufs=1))
        gpool = ctx.enter_context(tc.tile_pool(name="g", bufs=3))
        g2pool = ctx.enter_context(tc.tile_pool(name="g2", bufs=3))
        spool = ctx.enter_context(tc.tile_pool(name="s", bufs=4))
        fpool = ctx.enter_context(tc.tile_pool(name="f", bufs=3))
        psacc = ctx.enter_context(tc.tile_pool(name="psacc", bufs=2, space="PSUM"))
        psw = ctx.enter_context(tc.tile_pool(name="psw", bufs=2, space="PSUM"))

        # ---- constants
        iota_i = const.tile([128, 128], i32)
        nc.gpsimd.iota(iota_i[:], pattern=[[1, 128]], base=0, channel_multiplier=0)
        iota_f = const.tile([128, 128], f32)
        nc.vector.tensor_copy(iota_f[:], iota_i[:])
        ident = const.tile([128, 128], f32)
        make_identity(nc, ident[:])
        w1_sb = const.tile([F, F], f32)
        nc.sync.dma_start(w1_sb[:], w1_ext[:, :])
        w2_sb = const.tile([F, C], f32)
        nc.sync.dma_start(w2_sb[:], w2_ext[:, :])
        b1_sb = const.tile([F, 1], f32)
        nc.sync.dma_start(b1_sb[:], b1_ext[:, None])
        ones1 = const.tile([1, 128], f32)
        nc.vector.memset(ones1[:], 1.0)
        b2row = const.tile([1, C], f32)
        nc.sync.dma_start(b2row[:], b2_ext[None, :])
        b2ps = psw.tile([128, C], f32, tag="o2")
        nc.tensor.matmul(b2ps[:], lhsT=ones1[:], rhs=b2row[:], start=True, stop=True)
        b2b = const.tile([128, C], f32)
        nc.vector.tensor_copy(b2b[:], b2ps[:])

        idx_sb = meta.tile([128, IDX_COLS], i16)
        nc.sync.dma_start(idx_sb[:], idx_ext[:, :])
        ld_sb = meta.tile([128, slot_cols], f32)
        nc.sync.dma_start(ld_sb[:], ld_ext[:, :])
        q_sb = meta.tile([128, slot_cols], f32)
        nc.sync.dma_start(q_sb[:], q_ext[:, :])

        # =========================== layer 1 ===========================
        for w in range(WPC):
            gt = gpool.tile([128, SLOTS, F], f32, tag="g")
            if w < 3:
                nc.vector.memset(gt[:], 0.0)
            for c in range(NCH):
                g = w * NCH + c
                tg = tgs[g]
                if tg == 0:
                    continue
                nc.gpsimd.dma_gather(
                    gt[:, c * tcs: c * tcs + tg, :],
                    x_ext[c * CHUNK:(c + 1) * CHUNK, :],
                    idx_sb[:, g * idxb: g * idxb + tg * 8],
                    tg * 128,
                    int(cpad[g]),
                    F,
                )
            slots = [(c, t) for c in range(NCH) for t in range(tgs[w * NCH + c])]
            if not slots:
                continue
            acc = psacc.tile([128, 128], f32, tag="acc")
            for i, (c, t) in enumerate(slots):
                g = w * NCH + c
                scol = g * tcs + t
                s_t = spool.tile([128, 128], f32, tag="s")
                nc.vector.tensor_scalar(
                    out=s_t[:], in0=iota_f[:],
                    scalar1=ld_sb[:, scol:scol + 1],
                    scalar2=q_sb[:, scol:scol + 1],
                    op0=eq, op1=mul,
                )
                # acc[f, dst] += G[e, f].T @ S[e, dst]
                nc.tensor.matmul(acc[:], lhsT=gt[:, c * tcs + t, :], rhs=s_t[:],
                                 start=(i == 0), stop=(i == len(slots) - 1))
            # flush: out1.T = W1.T @ acc ; h.T = relu(out1.T + b1)
            ptsb = fpool.tile([128, 128], f32, tag="pt")
            nc.vector.tensor_copy(ptsb[:], acc[:])
            o1 = psw.tile([128, 128], f32, tag="o1")
            nc.tensor.matmul(o1[:], lhsT=w1_sb[:], rhs=ptsb[:], start=True, stop=True)
            ht = fpool.tile([128, 128], f32, tag="ht")
            nc.scalar.activation(ht[:], o1[:], Relu, bias=b1_sb[:, 0:1])
            o2 = psw.tile([C, 128], f32, tag="o2")
            nc.tensor.matmul(o2[:], lhsT=w2_sb[:], rhs=ht[:], start=True, stop=True)
            o2sb = fpool.tile([C, 128], f32, tag="o2sb")
            nc.vector.tensor_copy(o2sb[:], o2[:])
            hwp = psw.tile([128, C], f32, tag="tr")
            nc.tensor.transpose(hwp[:], o2sb[:], ident[0:C, 0:C])
            hwsb = fpool.tile([128, HWC], f32, tag="hw")
            if w < 3:
                nc.vector.memset(hwsb[:, C:HWC], 0.0)
            nc.vector.tensor_copy(hwsb[:, 0:C], hwp[:])
            nc.sync.dma_start(hw_own[w * 128:(w + 1) * 128, :], hwsb[:])

        # ======================= exchange hw table ======================
        nc.gpsimd.collective_compute(
            "AllGather", mybir.AluOpType.bypass,
            replica_groups=[list(range(CORES))],
            ins=[hw_own[:]], outs=[hw_tab[:]],
        )

        # =========================== layer 2 ===========================
        for w in range(WPC):
            g2 = g2pool.tile([128, SLOTS, HWC], f32, tag="g2")
            if w < 3:
                nc.vector.memset(g2[:], 0.0)
            for c in range(NCH):
                g = w * NCH + c
                tg = tgs[g]
                if tg == 0:
                    continue
                nc.gpsimd.dma_gather(
                    g2[:, c * tcs: c * tcs + tg, :],
                    hw_tab[c * CHUNK:(c + 1) * CHUNK, :],
                    idx_sb[:, g * idxb: g * idxb + tg * 8],
                    tg * 128,
                    int(cpad[g]),
                    HWC,
                )
            slots = [(c, t) for c in range(NCH) for t in range(tgs[w * NCH + c])]
            if not slots:
                continue
            acc2 = psacc.tile([128, HWC], f32, tag="acc")
            for i, (c, t) in enumerate(slots):
                g = w * NCH + c
                scol = g * tcs + t
                s_t = spool.tile([128, 128], f32, tag="s")
                nc.vector.tensor_scalar(
                    out=s_t[:], in0=iota_f[:],
                    scalar1=ld_sb[:, scol:scol + 1],
                    scalar2=q_sb[:, scol:scol + 1],
                    op0=eq, op1=mul,
                )
                # acc2[dst, hwc] += S[e, dst].T @ G2[e, hwc]
                nc.tensor.matmul(acc2[:], lhsT=s_t[:], rhs=g2[:, c * tcs + t, :],
                                 start=(i == 0), stop=(i == len(slots) - 1))
            # softmax(acc2[:, :C] + b2) along free dim
            t0 = fpool.tile([128, C], f32, tag="t0")
            nc.vector.tensor_tensor(out=t0[:], in0=acc2[:, 0:C], in1=b2b[:], op=add)
            mxn = fpool.tile([128, 1], f32, tag="mx")
            nc.vector.tensor_reduce(out=mxn[:], in_=t0[:], axis=AX, op=mx_op, negate=True)
            ex = fpool.tile([128, C], f32, tag="ex")
            nc.scalar.activation(ex[:], t0[:], Exp, bias=mxn[:, 0:1])
            sm = fpool.tile([128, 1], f32, tag="sm")
            nc.vector.tensor_reduce(out=sm[:], in_=ex[:], axis=AX, op=add)
            rc = fpool.tile([128, 1], f32, tag="rc")
            nc.vector.reciprocal(rc[:], sm[:])
            ot = fpool.tile([128, C], f32, tag="ot")
            nc.vector.tensor_scalar(out=ot[:], in0=ex[:], scalar1=rc[:, 0:1],
                                    scalar2=None, op0=mul)
            nc.sync.dma_start(out_ext[w * 128:(w + 1) * 128, :], ot[:])

    if not nc.is_finalized():
        nc.finalize()
    return nc


def _install_ntff_hook():
    """Profiling-only shim: some images lack antenv.axon_hooks; synthesize it
    from the injected libaxon so trace=True yields NTFF exec times."""
    import sys
    import types
    try:
        import antenv.axon_hooks  # noqa: F401
        return
    except ImportError:
        pass
    try:
        from trn_agent_boot.trn_boot import _ntff_profile_via_ctypes
        hook = _ntff_profile_via_ctypes("/opt/axon/libaxon_pjrt.so")
    except Exception:
        hook = None
    mod = types.ModuleType("antenv.axon_hooks")
    mod.get_axon_ntff_profile_hook = lambda: hook
    mod.set_axon_ntff_profile_hook = lambda h: None
    sys.modules["antenv.axon_hooks"] = mod
    # artifact upload has no destination in this sandbox; keep traces local
    import concourse.bass_utils as bu
    bu.upload_artifacts = lambda tmpdir: tmpdir


# -------------------------------------------------------------------- kernel
def kernel(x, edge_index, edge_weight, attention, W1, b1, W2, b2):
    from concourse.bass_utils import run_bass_kernel_spmd

    x_pad, per_core, cpad, tcs, slot_cols, idxb = _host_prep(x, edge_index, edge_weight)

    key = (tcs, tuple(int(v) for v in cpad))
    if key not in _CACHE:
        _CACHE.clear()
        _CACHE[key] = _build(cpad, tcs, slot_cols, idxb)
    nc = _CACHE[key]

    W1 = np.ascontiguousarray(np.asarray(W1, dtype=np.float32))
    b1 = np.ascontiguousarray(np.asarray(b1, dtype=np.float32))
    W2 = np.ascontiguousarray(np.asarray(W2, dtype=np.float32))
    b2 = np.ascontiguousarray(np.asarray(b2, dtype=np.float32))

    in_maps = []
    for p in range(CORES):
        in_maps.append({
            "x": x_pad,
            "idx": per_core[p]["idx"],
            "ld": per_core[p]["ld"],
            "q": per_core[p]["q"],
            "W1": W1, "b1": b1, "W2": W2, "b2": b2,
        })

    import os
    trace = bool(os.environ.get("GCN_TRACE"))
    if trace:
        _install_ntff_hook()
    res = run_bass_kernel_spmd(nc, in_maps, core_ids=list(range(CORES)),
                               trace=trace)
    if trace:
        print(f"HW exec time: {res.exec_time_ns} ns")
        kernel.last_exec_time_ns = res.exec_time_ns
        kernel.last_results = res
    out = np.concatenate([res.results[p]["out"] for p in range(CORES)], axis=0)
    return np.ascontiguousarray(out[:N])


if __name__ == "__main__":
    import reference
    inputs = reference.setup_inputs()
    got = kernel(**{k: np.asarray(v) for k, v in inputs.items()})
    print("kernel output", got.shape, got.dtype)


# revision 16
# speedup vs baseline: 2.6099x; 2.6099x over previous
"""Trainium2 Bass kernel for a 2-layer GCN with attention-weighted periods.

Reference computation (PERIODS=1, so softmax(attention)=1.0 exactly):
    h  = relu(gcn_conv(x, W1, b1))
    out = softmax(gcn_conv(h, W2, b2), axis=1)
where gcn_conv adds self-loops (weight 1), applies symmetric normalization
norm_e = dinv[src]*w_e*dinv[dst] with deg[j] = 1 + sum_{e->j} w_e, and
aggregates  out[d] = sum_e norm_e * (x @ W)[src_e] + b.

Key algebraic facts exploited:
  * aggregation commutes with the dense matmul:
        sum_e norm_e * (x@W)[src] = (sum_e norm_e * x[src]) @ W
    so the gather tables are the raw inputs (x for layer 1, h@W2 for layer 2)
    and the dense matmuls run once per 128 destination nodes.
  * both layers share the same normalized adjacency, computed once on the
    host as graph preprocessing (cached gcn_norm), and the same edge
    partitioning metadata.

Distribution (8 NeuronCores): nodes are sharded contiguously (12544/core);
edges are partitioned by destination core and grouped into 128-destination
"windows" that accumulate in PSUM via one-hot-valued matmuls
(S[e, d] = norm_e * (local_dst[e] == d)); source features are fetched with
dma_gather from a replicated table. Layer 2's table (h@W2, padded to 64
cols) is exchanged with an on-device AllGather.
"""

import math
import numpy as np

# ---------------------------------------------------------------- problem cfg
N = 100000          # real nodes
F = 128             # input/hidden features
C = 40              # classes
CORES = 8
NPC = 12544         # nodes per core (multiple of 128)
NPAD = CORES * NPC  # 100352
WIN = 64            # destinations per PSUM window
WPC = NPC // WIN    # windows per core = 196
NCH = 4             # source chunks (int16 gather index limit: 32767 rows)
CHUNK = NPAD // NCH  # 25088 rows per chunk
HWC = 128           # layer-2 table row width, bf16 (256B gather rows)
QUANT = 64          # quantize per-call index counts (register-value reuse)

_CACHE = {}


def _ceil_to(a, m):
    return -(-a // m) * m


# ------------------------------------------------------------------ host prep
def _host_prep(x, edge_index, edge_weight):
    """Builds per-core gather/one-hot metadata from the edge list.

    Returns (x_pad, per_core list of dicts, cpad [G] int array, tcs).
    """
    row = np.asarray(edge_index[0], dtype=np.int64)
    col = np.asarray(edge_index[1], dtype=np.int64)
    w = np.asarray(edge_weight, dtype=np.float64)

    deg = np.bincount(col, weights=w, minlength=N) + 1.0
    dinv = 1.0 / np.sqrt(deg)
    coef = dinv[row] * w * dinv[col]

    loop = np.arange(N, dtype=np.int64)
    arow = np.concatenate([row, loop])
    acol = np.concatenate([col, loop])
    acoef = np.concatenate([coef, dinv * dinv]).astype(np.float32)

    core = acol // NPC
    G = WPC * NCH

    per_core = []
    counts = np.zeros((CORES, G), dtype=np.int64)
    for p in range(CORES):
        sel = core == p
        r = arow[sel]
        cl = acol[sel] - p * NPC
        q = acoef[sel]
        wid = cl // WIN
        ld = (cl % WIN).astype(np.float32)
        ch = r // CHUNK
        li = (r - ch * CHUNK).astype(np.int64)
        grp = wid * NCH + ch
        order = np.lexsort((r, grp))
        grp, ld, li, q = grp[order], ld[order], li[order], q[order]
        counts[p] = np.bincount(grp, minlength=G)
        per_core.append({"grp": grp, "ld": ld, "li": li, "q": q})

    cmax = counts.max(axis=0)
    cpad = np.minimum(_ceil_to(1, QUANT) * 0 + ((cmax + QUANT - 1) // QUANT) * QUANT,
                      10 ** 9)
    tcs = max(1, int(-(-int(cpad.max()) // 128)))
    cap = tcs * 128
    assert (cpad <= cap).all()

    idxb = tcs * 8          # idx columns per group
    slot_cols = G * tcs     # ld/q columns

    for p in range(CORES):
        d = per_core[p]
        grp, ld, li, q = d["grp"], d["ld"], d["li"], d["q"]
        cnt = counts[p]
        starts = np.zeros(G + 1, dtype=np.int64)
        np.cumsum(cnt, out=starts[1:])
        pos = np.arange(len(grp)) - starts[grp]

        idx_arr = np.full((G, cap), -1, dtype=np.int16)
        idx_arr[grp, pos] = li.astype(np.int16)
        # keep at least one valid index in non-empty (globally) groups so the
        # gather ucode/sim never sees an all-(-1) index stream
        empty = (cnt == 0) & (cpad > 0)
        idx_arr[empty, 0] = 0
        cnt_eff = np.maximum(cnt, (cpad > 0).astype(np.int64)).astype(np.int32)

        ld_arr = np.zeros((G, cap), dtype=np.float32)
        q_arr = np.zeros((G, cap), dtype=np.float32)
        ld_arr[grp, pos] = ld
        q_arr[grp, pos] = q

        idx_dev = idx_arr.reshape(G, idxb, 16).transpose(2, 0, 1).reshape(16, G * idxb)
        idx_dev = np.ascontiguousarray(np.tile(idx_dev, (8, 1)))
        ld_dev = np.ascontiguousarray(ld_arr.reshape(G * tcs, 128).T)
        q_dev = np.ascontiguousarray(q_arr.reshape(G * tcs, 128).T)
        import ml_dtypes
        per_core[p] = {"idx": idx_dev,
                       "ld": ld_dev.astype(ml_dtypes.bfloat16),
                       "q": q_dev.astype(ml_dtypes.bfloat16),
                       "cnt": np.ascontiguousarray(cnt_eff[None, :])}

    import ml_dtypes
    x = np.asarray(x, dtype=np.float32)
    x_pad = np.zeros((NPAD, F), dtype=ml_dtypes.bfloat16)
    x_pad[:N] = x.astype(ml_dtypes.bfloat16)
    return x_pad, per_core, cpad.astype(np.int64), tcs, slot_cols, idxb


# ------------------------------------------------------------------- builder
def _build(cpad, tcs, slot_cols, idxb):
    from contextlib import ExitStack
    import concourse.bacc as bacc
    import concourse.tile as tile
    import concourse.mybir as mybir
    from concourse.masks import make_identity

    f32 = mybir.dt.float32
    bf16 = mybir.dt.bfloat16
    i16 = mybir.dt.int16
    i32 = mybir.dt.int32
    G = WPC * NCH
    IDX_COLS = G * idxb
    SLOTS = NCH * tcs

    nc = bacc.Bacc("TRN2", target_bir_lowering=False, num_swdge_queues=4)

    x_ext = nc.declare_dram_parameter("x", [NPAD, F], bf16, isOutput=False)
    idx_ext = nc.declare_dram_parameter("idx", [128, IDX_COLS], i16, isOutput=False)
    ld_ext = nc.declare_dram_parameter("ld", [128, slot_cols], bf16, isOutput=False)
    q_ext = nc.declare_dram_parameter("q", [128, slot_cols], bf16, isOutput=False)
    cnt_ext = nc.declare_dram_parameter("cnt", [1, G], i32, isOutput=False)
    w1_ext = nc.declare_dram_parameter("W1", [F, F], bf16, isOutput=False)
    b1_ext = nc.declare_dram_parameter("b1", [F], f32, isOutput=False)
    w2_ext = nc.declare_dram_parameter("W2", [F, C], bf16, isOutput=False)
    b2_ext = nc.declare_dram_parameter("b2", [C], f32, isOutput=False)
    out_ext = nc.declare_dram_parameter("out", [NPC, C], f32, isOutput=True)

    hw_own = nc.dram_tensor("hw_own", [NPC, HWC], bf16)
    hw_tab = nc.dram_tensor("hw_tab", [NPAD, HWC], bf16, addr_space="Shared")

    tgs = [max(0, -(-int(cpad[g]) // 128)) if cpad[g] > 0 else 0 for g in range(G)]

    eq = mybir.AluOpType.is_equal
    mul = mybir.AluOpType.mult
    add = mybir.AluOpType.add
    mx_op = mybir.AluOpType.max
    AX = mybir.AxisListType.X
    Relu = mybir.ActivationFunctionType.Relu
    Exp = mybir.ActivationFunctionType.Exp

    with ExitStack() as ctx:
        tc = ctx.enter_context(tile.TileContext(nc))
        const = ctx.enter_context(tc.tile_pool(name="const", bufs=1))
        meta = ctx.enter_context(tc.tile_pool(name="meta", bufs=1))
        gpool = ctx.enter_context(tc.tile_pool(name="g", bufs=3))
        g2pool = ctx.enter_context(tc.tile_pool(name="g2", bufs=3))
        spool = ctx.enter_context(tc.tile_pool(name="s", bufs=2))
        fpool = ctx.enter_context(tc.tile_pool(name="f", bufs=3))
        psacc = ctx.enter_context(tc.tile_pool(name="psacc", bufs=2, space="PSUM"))
        psw = ctx.enter_context(tc.tile_pool(name="psw", bufs=2, space="PSUM"))

        # ---- constants
        iota_i = const.tile([128, tcs, WIN], i32)
        nc.gpsimd.iota(iota_i[:], pattern=[[0, tcs], [1, WIN]], base=0,
                       channel_multiplier=0)
        iota_w = const.tile([128, tcs, WIN], bf16)
        nc.vector.tensor_copy(iota_w[:], iota_i[:])
        ident = const.tile([128, 128], bf16)
        make_identity(nc, ident[:])
        w1_sb = const.tile([F, F], bf16)
        nc.sync.dma_start(w1_sb[:], w1_ext[:, :])
        w2_sb = const.tile([F, C], bf16)
        nc.sync.dma_start(w2_sb[:], w2_ext[:, :])
        b1_sb = const.tile([F, 1], f32)
        nc.sync.dma_start(b1_sb[:], b1_ext[:, None])
        ones1 = const.tile([1, 128], f32)
        nc.vector.memset(ones1[:], 1.0)
        b2row = const.tile([1, C], f32)
        nc.sync.dma_start(b2row[:], b2_ext[None, :])
        b2ps = psw.tile([128, C], f32, tag="o2")
        nc.tensor.matmul(b2ps[:], lhsT=ones1[:], rhs=b2row[:], start=True, stop=True)
        b2b = const.tile([128, C], f32)
        nc.vector.tensor_copy(b2b[:], b2ps[:])

        idx_sb = meta.tile([128, IDX_COLS], i16)
        nc.sync.dma_start(idx_sb[:], idx_ext[:, :])
        ld_sb = meta.tile([128, slot_cols], bf16)
        nc.sync.dma_start(ld_sb[:], ld_ext[:, :])
        q_sb = meta.tile([128, slot_cols], bf16)
        nc.sync.dma_start(q_sb[:], q_ext[:, :])
        cnt_sb = meta.tile([1, G], i32)
        nc.sync.dma_start(cnt_sb[:], cnt_ext[:, :])
        nreg = nc.gpsimd.alloc_register("nidx")

        # =========================== layer 1 ===========================
        for w in range(WPC):
            gt = gpool.tile([128, SLOTS, F], bf16, tag="g")
            if w < 3:
                nc.vector.memset(gt[:], 0.0)
            for c in range(NCH):
                g = w * NCH + c
                tg = tgs[g]
                if tg == 0:
                    continue
                nc.gpsimd.reg_load(nreg, cnt_sb[0:1, g:g + 1])
                nc.gpsimd.dma_gather(
                    gt[:, c * tcs: c * tcs + tg, :],
                    x_ext[c * CHUNK:(c + 1) * CHUNK, :],
                    idx_sb[:, g * idxb: g * idxb + tg * 8],
                    tg * 128,
                    nreg,
                    F,
                    queue_num=c % 4,
                )
            slots = [(c, t) for c in range(NCH) for t in range(tgs[w * NCH + c])]
            if not slots:
                continue
            sw = spool.tile([128, SLOTS, WIN], bf16, tag="s")
            for c in range(NCH):
                g = w * NCH + c
                tg = tgs[g]
                if tg == 0:
                    continue
                sl = sw[:, c * tcs: c * tcs + tg, :]
                nc.vector.tensor_tensor(
                    out=sl, in0=iota_w[:, 0:tg, :],
                    in1=ld_sb[:, g * tcs: g * tcs + tg].to_broadcast([128, tg, WIN]),
                    op=eq)
                nc.vector.tensor_tensor(
                    out=sl, in0=sl,
                    in1=q_sb[:, g * tcs: g * tcs + tg].to_broadcast([128, tg, WIN]),
                    op=mul)
            acc = psacc.tile([128, WIN], f32, tag="acc")
            for i, (c, t) in enumerate(slots):
                # acc[f, dst] += G[e, f].T @ S[e, dst]
                nc.tensor.matmul(acc[:], lhsT=gt[:, c * tcs + t, :],
                                 rhs=sw[:, c * tcs + t, :],
                                 start=(i == 0), stop=(i == len(slots) - 1))
            # flush: out1.T = W1.T @ acc ; h.T = relu(out1.T + b1)
            ptsb = fpool.tile([128, WIN], bf16, tag="pt")
            nc.vector.tensor_copy(ptsb[:], acc[:])
            o1 = psw.tile([128, WIN], f32, tag="o1")
            nc.tensor.matmul(o1[:], lhsT=w1_sb[:], rhs=ptsb[:], start=True, stop=True)
            ht = fpool.tile([128, WIN], bf16, tag="ht")
            nc.scalar.activation(ht[:], o1[:], Relu, bias=b1_sb[:, 0:1])
            o2 = psw.tile([C, WIN], f32, tag="o2")
            nc.tensor.matmul(o2[:], lhsT=w2_sb[:], rhs=ht[:], start=True, stop=True)
            o2sb = fpool.tile([C, WIN], bf16, tag="o2sb")
            nc.scalar.copy(o2sb[:], o2[:])
            hwp = psw.tile([WIN, C], bf16, tag="tr")
            nc.tensor.transpose(hwp[:], o2sb[:], ident[0:C, 0:C])
            hwsb = fpool.tile([WIN, HWC], bf16, tag="hw")
            if w < 3:
                nc.vector.memset(hwsb[:, C:HWC], 0.0)
            nc.vector.tensor_copy(hwsb[:, 0:C], hwp[:])
            nc.sync.dma_start(hw_own[w * WIN:(w + 1) * WIN, :], hwsb[:])

        # ======================= exchange hw table ======================
        nc.gpsimd.collective_compute(
            "AllGather", mybir.AluOpType.bypass,
            replica_groups=[list(range(CORES))],
            ins=[hw_own[:]], outs=[hw_tab[:]],
        )

        # =========================== layer 2 ===========================
        for w in range(WPC):
            g2 = g2pool.tile([128, SLOTS, HWC], bf16, tag="g2")
            if w < 3:
                nc.vector.memset(g2[:], 0.0)
            for c in range(NCH):
                g = w * NCH + c
                tg = tgs[g]
                if tg == 0:
                    continue
                nc.gpsimd.reg_load(nreg, cnt_sb[0:1, g:g + 1])
                nc.gpsimd.dma_gather(
                    g2[:, c * tcs: c * tcs + tg, :],
                    hw_tab[c * CHUNK:(c + 1) * CHUNK, :],
                    idx_sb[:, g * idxb: g * idxb + tg * 8],
                    tg * 128,
                    nreg,
                    HWC,
                    queue_num=c % 4,
                )
            slots = [(c, t) for c in range(NCH) for t in range(tgs[w * NCH + c])]
            if not slots:
                continue
            sw = spool.tile([128, SLOTS, WIN], bf16, tag="s")
            for c in range(NCH):
                g = w * NCH + c
                tg = tgs[g]
                if tg == 0:
                    continue
                sl = sw[:, c * tcs: c * tcs + tg, :]
                nc.vector.tensor_tensor(
                    out=sl, in0=iota_w[:, 0:tg, :],
                    in1=ld_sb[:, g * tcs: g * tcs + tg].to_broadcast([128, tg, WIN]),
                    op=eq)
                nc.vector.tensor_tensor(
                    out=sl, in0=sl,
                    in1=q_sb[:, g * tcs: g * tcs + tg].to_broadcast([128, tg, WIN]),
                    op=mul)
            acc2 = psacc.tile([WIN, 64], f32, tag="acc")
            for i, (c, t) in enumerate(slots):
                # acc2[dst, hwc] += S[e, dst].T @ G2[e, hwc]
                nc.tensor.matmul(acc2[:], lhsT=sw[:, c * tcs + t, :],
                                 rhs=g2[:, c * tcs + t, 0:64],
                                 start=(i == 0), stop=(i == len(slots) - 1))
            # softmax(acc2[:, :C] + b2) along free dim
            t0 = fpool.tile([WIN, C], f32, tag="t0")
            nc.vector.tensor_tensor(out=t0[:], in0=acc2[:, 0:C], in1=b2b[0:WIN, :],
                                    op=add)
            mxn = fpool.tile([WIN, 1], f32, tag="mx")
            nc.vector.tensor_reduce(out=mxn[:], in_=t0[:], axis=AX, op=mx_op, negate=True)
            ex = fpool.tile([WIN, C], f32, tag="ex")
            nc.scalar.activation(ex[:], t0[:], Exp, bias=mxn[:, 0:1])
            sm = fpool.tile([WIN, 1], f32, tag="sm")
            nc.vector.tensor_reduce(out=sm[:], in_=ex[:], axis=AX, op=add)
            rc = fpool.tile([WIN, 1], f32, tag="rc")
            nc.vector.reciprocal(rc[:], sm[:])
            ot = fpool.tile([WIN, C], f32, tag="ot")
            nc.scalar.activation(ot[:], ex[:],
                                 mybir.ActivationFunctionType.Copy,
                                 scale=rc[:, 0:1])
            nc.sync.dma_start(out_ext[w * WIN:(w + 1) * WIN, :], ot[:])

    if not nc.is_finalized():
        nc.finalize()
    return nc


def _install_ntff_hook():
    """Profiling-only shim: some images lack antenv.axon_hooks; synthesize it
    from the injected libaxon so trace=True yields NTFF exec times."""
    import sys
    import types
    try:
        import antenv.axon_hooks  # noqa: F401
        return
    except ImportError:
        pass
    try:
        from trn_agent_boot.trn_boot import _ntff_profile_via_ctypes
        hook = _ntff_profile_via_ctypes("/opt/axon/libaxon_pjrt.so")
    except Exception:
        hook = None
    mod = types.ModuleType("antenv.axon_hooks")
    mod.get_axon_ntff_profile_hook = lambda: hook
    mod.set_axon_ntff_profile_hook = lambda h: None
    sys.modules["antenv.axon_hooks"] = mod
    # artifact upload has no destination in this sandbox; keep traces local
    import concourse.bass_utils as bu
    bu.upload_artifacts = lambda tmpdir: tmpdir


# -------------------------------------------------------------------- kernel
def kernel(x, edge_index, edge_weight, attention, W1, b1, W2, b2):
    from concourse.bass_utils import run_bass_kernel_spmd

    x_pad, per_core, cpad, tcs, slot_cols, idxb = _host_prep(x, edge_index, edge_weight)

    key = (tcs, tuple(int(v) for v in cpad))
    if key not in _CACHE:
        _CACHE.clear()
        _CACHE[key] = _build(cpad, tcs, slot_cols, idxb)
    nc = _CACHE[key]

    import ml_dtypes
    W1 = np.ascontiguousarray(np.asarray(W1, dtype=np.float32).astype(ml_dtypes.bfloat16))
    b1 = np.ascontiguousarray(np.asarray(b1, dtype=np.float32))
    W2 = np.ascontiguousarray(np.asarray(W2, dtype=np.float32).astype(ml_dtypes.bfloat16))
    b2 = np.ascontiguousarray(np.asarray(b2, dtype=np.float32))

    in_maps = []
    for p in range(CORES):
        in_maps.append({
            "x": x_pad,
            "idx": per_core[p]["idx"],
            "ld": per_core[p]["ld"],
            "q": per_core[p]["q"],
            "cnt": per_core[p]["cnt"],
            "W1": W1, "b1": b1, "W2": W2, "b2": b2,
        })

    import os
    trace = bool(os.environ.get("GCN_TRACE"))
    if trace:
        _install_ntff_hook()
    res = run_bass_kernel_spmd(nc, in_maps, core_ids=list(range(CORES)),
                               trace=trace)
    if trace:
        print(f"HW exec time: {res.exec_time_ns} ns")
        kernel.last_exec_time_ns = res.exec_time_ns
        kernel.last_results = res
    out = np.concatenate([res.results[p]["out"] for p in range(CORES)], axis=0)
    return np.ascontiguousarray(out[:N])


if __name__ == "__main__":
    import reference
    inputs = reference.setup_inputs()
    got = kernel(**{k: np.asarray(v) for k, v in inputs.items()})
    print("kernel output", got.shape, got.dtype)
